# revision 1
# baseline (speedup 1.0000x reference)
"""Trainium2 Bass kernel for nn_MultiHeadAttention (dense transformer block:
qkv proj + RoPE + causal SDPA + out proj), tensor-parallel over (batch, heads)
across 8 NeuronCores.

Sharding: 2 batches x 16 heads = 32 (b,h) pairs; core c handles batch c//4,
heads 4*(c%4)..4*(c%4)+3. Each core computes qkv for its 4 heads (from the
full x of its batch), RoPE, causal attention, and a PARTIAL output
projection (its heads' rows of Wproj); the host sums the 4 partials per
batch. All matmuls run in bf16 (fp32 PSUM accumulation).

Layout notes:
- x is passed pre-transposed per batch (xT [D, S]) so the contraction dim
  (model dim) lands on SBUF partitions with no on-device transpose.
- q/k head dims are permuted host-side into a 16-interleaved (even,odd)
  order so RoPE's pair swap is a quadrant-local DVE stream_shuffle.
  Attention scores are invariant to this (q and k permuted identically).
- Scores are computed transposed (S^T [kv, q]) so softmax's denominator
  comes from a ones-matmul (column sums) and P^T feeds the O = V^T @ P^T
  matmul directly. exp() runs without max-subtraction: |scores| < ~10 for
  this input distribution, safe in fp32.
"""
import sys

sys.path.insert(0, "/opt/trn_rl_repo")

import numpy as np
import ml_dtypes

import concourse.bass as bass
import concourse.mybir as mybir
import concourse.tile as tile

P = 128
B, S, D = 2, 2048, 2048
NH, HD = 16, 128
NH_CORE = 4  # heads per core
HCOLS = NH_CORE * HD  # 512
KT = D // P  # 16 k-tiles
TT = S // P  # 16 token tiles
QC = 512  # q-chunk width
NQC = S // QC  # 4
ROPE_THETA = 10000.0
SCALE = HD**-0.5
NEG = -30000.0

F32 = mybir.dt.float32
BF16 = mybir.dt.bfloat16

_SWAP16 = [(i + 16) % 32 for i in range(32)]


# ---------------------------------------------------------------------------
# host-side constant tables
# ---------------------------------------------------------------------------
def _dim_perm():
    """Permutation p -> original head-dim index, 16-interleaved even/odd."""
    perm = np.zeros(HD, dtype=np.int64)
    for p in range(HD):
        qd, sl = p // 32, p % 32
        i = 16 * qd + (sl % 16)
        perm[p] = 2 * i if sl < 16 else 2 * i + 1
    return perm


def _rope_tables():
    """ctab[p,t], stab[p,t] (sign-baked) for the permuted head-dim layout."""
    perm = _dim_perm()
    inv_freq = 1.0 / (ROPE_THETA ** (np.arange(0, HD, 2, dtype=np.float64) / HD))
    t = np.arange(S, dtype=np.float64)
    ctab = np.zeros((HD, S), dtype=np.float64)
    stab = np.zeros((HD, S), dtype=np.float64)
    for p in range(HD):
        qd, sl = p // 32, p % 32
        i = 16 * qd + (sl % 16)
        ang = t * inv_freq[i]
        ctab[p] = np.cos(ang)
        stab[p] = -np.sin(ang) if sl < 16 else np.sin(ang)
    return ctab.astype(np.float32), stab.astype(np.float32)


def _tri_mask():
    """[P, P] f32: 0 where kv(row) <= q(col) else NEG."""
    b = np.arange(P)[:, None]
    a = np.arange(P)[None, :]
    return np.where(b <= a, 0.0, NEG).astype(np.float32)


# ---------------------------------------------------------------------------
# device kernel
# ---------------------------------------------------------------------------
def _build_nc():
    nc = bass.Bass()

    xT = nc.declare_dram_parameter("xT", [D, S], BF16, isOutput=False)
    Wq = nc.declare_dram_parameter("Wq", [D, HCOLS], BF16, isOutput=False)
    Wk = nc.declare_dram_parameter("Wk", [D, HCOLS], BF16, isOutput=False)
    Wv = nc.declare_dram_parameter("Wv", [D, HCOLS], BF16, isOutput=False)
    Wp = nc.declare_dram_parameter("Wp", [HCOLS, D], BF16, isOutput=False)
    out = nc.declare_dram_parameter("out", [S, D], F32, isOutput=True)

    # sqrt(SCALE) on both q and k tables => scores scaled by SCALE
    ctab_np, stab_np = _rope_tables()
    rt = np.sqrt(SCALE).astype(np.float32)
    cq_d = nc.inline_tensor((ctab_np * rt).astype(ml_dtypes.bfloat16), "cq")
    sq_d = nc.inline_tensor((stab_np * rt).astype(ml_dtypes.bfloat16), "sq")
    mask_d = nc.inline_tensor(_tri_mask(), "trimask")

    xT_t = xT[:].rearrange("(ko p) t -> p ko t", p=P)
    Wq_t = Wq[:].rearrange("(ko p) m -> p ko m", p=P)
    Wk_t = Wk[:].rearrange("(ko p) m -> p ko m", p=P)
    Wv_t = Wv[:].rearrange("(ko p) m -> p ko m", p=P)
    Wp_t = Wp[:].rearrange("(ho p) n -> p ho n", p=P)
    out_t = out[:].rearrange("(to p) n -> p to n", p=P)

    with tile.TileContext(nc) as tc:
        with (
            tc.tile_pool(name="persist", bufs=1) as pp,
            tc.tile_pool(name="work", bufs=2) as wk,
        ):
            # persistent tiles
            cq = pp.tile([P, S], BF16)
            sq = pp.tile([P, S], BF16)
            nc.sync.dma_start(cq, cq_d[:])
            nc.sync.dma_start(sq, sq_d[:])
            trimask = pp.tile([P, P], F32)
            nc.sync.dma_start(trimask, mask_d[:])
            ones_sb = pp.tile([P, P], BF16)
            nc.vector.memset(ones_sb, 1.0)

            Qt = pp.tile([P, NH_CORE, S], BF16)
            Kt = pp.tile([P, NH_CORE, S], BF16)
            Vt = pp.tile([P, TT, HCOLS], BF16)
            Yt = pp.tile([P, NH_CORE, S], BF16)

            # ---------------- phase 1: qkv projection + RoPE ---------------
            with (
                tc.tile_pool(name="mm1", bufs=1) as mm1p,
                tc.tile_pool(name="ps_mm1", bufs=8, space="PSUM") as psA,
            ):
                xT_sb = mm1p.tile([P, KT, S], BF16)
                Wq_sb = mm1p.tile([P, KT, HCOLS], BF16)
                Wk_sb = mm1p.tile([P, KT, HCOLS], BF16)
                Wv_sb = mm1p.tile([P, KT, HCOLS], BF16)
                for ki in range(KT):
                    nc.sync.dma_start(xT_sb[:, ki], xT_t[:, ki])
                    nc.gpsimd.dma_start(Wq_sb[:, ki], Wq_t[:, ki])
                    nc.gpsimd.dma_start(Wk_sb[:, ki], Wk_t[:, ki])
                    nc.gpsimd.dma_start(Wv_sb[:, ki], Wv_t[:, ki])

                # q and k projections with fused RoPE
                for W_sb, O_t, ctb, stb in (
                    (Wq_sb, Qt, cq, sq),
                    (Wk_sb, Kt, cq, sq),
                ):
                    for h in range(NH_CORE):
                        for tcx in range(NQC):
                            ps = psA.tile([P, QC], F32, tag="ps")
                            for ki in range(KT):
                                nc.tensor.matmul(
                                    ps,
                                    W_sb[:, ki, h * HD : (h + 1) * HD],
                                    xT_sb[:, ki, tcx * QC : (tcx + 1) * QC],
                                    start=(ki == 0),
                                    stop=(ki == KT - 1),
                                )
                            csl = ctb[:, tcx * QC : (tcx + 1) * QC]
                            ssl = stb[:, tcx * QC : (tcx + 1) * QC]
                            pc = wk.tile([P, QC], BF16, tag="pc")
                            nc.scalar.activation(
                                pc, ps, mybir.ActivationFunctionType.Copy
                            )
                            xsw = wk.tile([P, QC], BF16, tag="xsw")
                            nc.vector.stream_shuffle(xsw, pc, _SWAP16)
                            m1 = wk.tile([P, QC], BF16, tag="m1")
                            nc.vector.tensor_mul(m1, pc, csl)
                            m2 = wk.tile([P, QC], BF16, tag="m2")
                            nc.vector.tensor_mul(m2, xsw, ssl)
                            nc.vector.tensor_add(
                                O_t[:, h, tcx * QC : (tcx + 1) * QC], m1, m2
                            )

                # v projection (token-major)
                for tt in range(TT):
                    ps = psA.tile([P, HCOLS], F32, tag="ps")
                    for ki in range(KT):
                        nc.tensor.matmul(
                            ps,
                            xT_sb[:, ki, tt * P : (tt + 1) * P],
                            Wv_sb[:, ki],
                            start=(ki == 0),
                            stop=(ki == KT - 1),
                        )
                    nc.scalar.activation(
                        Vt[:, tt], ps, mybir.ActivationFunctionType.Copy
                    )

            # ---------------- phase 2: attention + out projection ----------
            with tc.tile_pool(name="attn", bufs=1) as atp:
                Wp_sb = atp.tile([P, NH_CORE, D], BF16)
                for ho in range(NH_CORE):
                    nc.sync.dma_start(Wp_sb[:, ho], Wp_t[:, ho])

                with (
                    tc.tile_pool(name="pt", bufs=3) as ptp,
                    tc.tile_pool(name="ps_o", bufs=2, space="PSUM") as psO,
                    tc.tile_pool(name="ps_s", bufs=5, space="PSUM") as psS,
                    tc.tile_pool(name="ps_l", bufs=1, space="PSUM") as psL,
                ):
                  for h in range(NH_CORE):
                    for qc in range(NQC):
                        o_ps = psO.tile([P, QC], F32, tag="ops")
                        l_ps = psL.tile([P, QC], F32, tag="lps")
                        njb = 4 * qc + 4
                        for jb in range(njb):
                            d = jb - 4 * qc  # diag offset if >= 0
                            off = 128 * d if d > 0 else 0
                            s_ps = psS.tile([P, QC], F32, tag="sps")
                            nc.tensor.matmul(
                                s_ps[:, off:],
                                Kt[:, h, jb * P : (jb + 1) * P],
                                Qt[:, h, qc * QC + off : (qc + 1) * QC],
                                start=True,
                                stop=True,
                            )
                            if d >= 0:
                                nc.vector.tensor_add(
                                    s_ps[:, off : off + P],
                                    s_ps[:, off : off + P],
                                    trimask,
                                )
                            pt = ptp.tile([P, QC], BF16, tag="pt")
                            nc.scalar.activation(
                                pt[:, off:],
                                s_ps[:, off:],
                                mybir.ActivationFunctionType.Exp,
                            )
                            nc.tensor.matmul(
                                o_ps[:, off:],
                                Vt[:, jb, h * HD : (h + 1) * HD],
                                pt[:, off:],
                                start=(jb == 0),
                                stop=(jb == njb - 1),
                            )
                            nc.tensor.matmul(
                                l_ps[:, off:],
                                ones_sb,
                                pt[:, off:],
                                start=(jb == 0),
                                stop=(jb == njb - 1),
                            )
                        rinv = wk.tile([P, QC], F32, tag="rinv")
                        nc.vector.reciprocal(rinv, l_ps)
                        nc.vector.tensor_mul(
                            Yt[:, h, qc * QC : (qc + 1) * QC], o_ps, rinv
                        )

                # out projection (partial; host sums across 4 cores/batch)
                with (
                    tc.tile_pool(name="outp", bufs=3) as outp,
                    tc.tile_pool(name="ps_p", bufs=6, space="PSUM") as psP,
                ):
                    for tt in range(TT):
                        ob = outp.tile([P, D], F32, tag="ob")
                        for ncx in range(D // QC):
                            ps = psP.tile([P, QC], F32, tag="psp")
                            for ho in range(NH_CORE):
                                nc.tensor.matmul(
                                    ps,
                                    Yt[:, ho, tt * P : (tt + 1) * P],
                                    Wp_sb[:, ho, ncx * QC : (ncx + 1) * QC],
                                    start=(ho == 0),
                                    stop=(ho == NH_CORE - 1),
                                )
                            nc.any.tensor_copy(
                                ob[:, ncx * QC : (ncx + 1) * QC], ps
                            )
                        nc.sync.dma_start(out_t[:, tt], ob)
    return nc


# ---------------------------------------------------------------------------
# legalization: this walrus build supports only ONE sync wait per instruction
# ---------------------------------------------------------------------------
_ENGINE_SEM_PREFIX = {
    "PE": "PE_",
    "DVE": "DVE_",
    "ACT": "ACT_",
    "Pool": "POOL_",
    "SP": "SP_",
}
_wf_counter = [0]


def _legalize(nc, max_waits=1):
    for f in nc.m.functions:
        for bb in f.blocks:
            new_insts = []
            for inst in bb.instructions:
                si = getattr(inst, "sync_info", None)
                eng = getattr(inst, "engine", None)
                if si is None or not si.on_wait or eng is None:
                    new_insts.append(inst)
                    continue
                waits = list(si.on_wait)
                pref = _ENGINE_SEM_PREFIX.get(eng.name)
                if pref is not None:
                    waits = [
                        w
                        for w in waits
                        if not (
                            w.sync_type == "semaphore"
                            and w.ant_name.startswith(pref)
                        )
                    ]
                if len(waits) > max_waits:
                    for w in waits[:-max_waits]:
                        _wf_counter[0] += 1
                        nop = mybir.InstNoOp(
                            name=f"I-waitfix-{_wf_counter[0]}", ins=[], outs=[]
                        )
                        nop.engine = eng
                        nop.sync_info = mybir.SyncInfo(on_wait=[w], on_update=[])
                        new_insts.append(nop)
                    waits = waits[-max_waits:]
                if len(waits) != len(si.on_wait):
                    inst.sync_info = mybir.SyncInfo(
                        on_wait=waits, on_update=list(si.on_update)
                    )
                new_insts.append(inst)
            bb.instructions[:] = new_insts


# ---------------------------------------------------------------------------
# SPMD runner (mirrors concourse.bass2jax.run_bass_via_pjrt, kept resident)
# ---------------------------------------------------------------------------
class _Runner:
    def __init__(self, nc, n_cores=8):
        import jax
        from jax.sharding import Mesh, PartitionSpec
        from jax.experimental.shard_map import shard_map
        from concourse import bass2jax
        from concourse.bass2jax import _bass_exec_p, install_neuronx_cc_hook

        install_neuronx_cc_hook()
        self.jax = jax
        self.nc = nc
        self.n_cores = n_cores
        partition_name = (
            nc.partition_id_tensor.name if nc.partition_id_tensor else None
        )
        in_names, out_names, out_avals, zero_outs = [], [], [], []
        for alloc in nc.m.functions[0].allocations:
            if not isinstance(alloc, mybir.MemoryLocationSet):
                continue
            name = alloc.memorylocations[0].name
            if alloc.kind == "ExternalInput":
                if name != partition_name:
                    in_names.append(name)
            elif alloc.kind == "ExternalOutput":
                shape = tuple(alloc.tensor_shape)
                dtype = mybir.dt.np(alloc.dtype)
                out_names.append(name)
                out_avals.append(jax.core.ShapedArray(shape, dtype))
                zero_outs.append(np.zeros(shape, dtype))
        self.in_names, self.out_names = in_names, out_names
        self.out_avals, self.zero_outs = out_avals, zero_outs
        n_params, n_outs = len(in_names), len(out_names)
        all_in_names = in_names + out_names
        if partition_name is not None:
            all_in_names.append(partition_name)
        donate = tuple(range(n_params, n_params + n_outs))

        def _body(*args):
            operands = list(args)
            if partition_name is not None:
                operands.append(bass2jax.partition_id_tensor())
            return tuple(
                _bass_exec_p.bind(
                    *operands,
                    out_avals=tuple(out_avals),
                    in_names=tuple(all_in_names),
                    out_names=tuple(out_names),
                    lowering_input_output_aliases=(),
                    sim_require_finite=True,
                    sim_require_nnan=True,
                    nc=nc,
                )
            )

        devices = jax.devices()[:n_cores]
        mesh = Mesh(np.asarray(devices), ("core",))
        in_specs = (PartitionSpec("core"),) * (n_params + n_outs)
        out_specs = (PartitionSpec("core"),) * n_outs
        self.fn = jax.jit(
            shard_map(
                _body,
                mesh=mesh,
                in_specs=in_specs,
                out_specs=out_specs,
                check_rep=False,
            ),
            donate_argnums=donate,
            keep_unused=True,
        )

    def run(self, in_maps):
        n = self.n_cores
        concat_in = [
            np.concatenate(
                [np.asarray(in_maps[c][name]) for c in range(n)], axis=0
            )
            for name in self.in_names
        ]
        zeros = [
            np.zeros((n * z.shape[0], *z.shape[1:]), z.dtype)
            for z in self.zero_outs
        ]
        out_arrs = self.fn(*concat_in, *zeros)
        return [
            {
                name: np.asarray(out_arrs[i]).reshape(
                    n, *self.out_avals[i].shape
                )[c]
                for i, name in enumerate(self.out_names)
            }
            for c in range(n)
        ]


_RUNNER = None


def _get_runner():
    global _RUNNER
    if _RUNNER is None:
        nc = _build_nc()
        _legalize(nc)
        _RUNNER = _Runner(nc, 8)
    return _RUNNER


# ---------------------------------------------------------------------------
# public entry point
# ---------------------------------------------------------------------------
def kernel(x, Wqkv, Wproj):
    x = np.asarray(x, dtype=np.float32)
    Wqkv = np.asarray(Wqkv, dtype=np.float32)
    Wproj = np.asarray(Wproj, dtype=np.float32)
    bf = ml_dtypes.bfloat16
    perm = _dim_perm()

    xT = [np.ascontiguousarray(x[b].T).astype(bf) for b in range(B)]
    in_maps = []
    for c in range(8):
        b, g = c // 4, c % 4
        heads = range(NH_CORE * g, NH_CORE * (g + 1))
        qcols = np.concatenate([h * HD + perm for h in heads])
        Wq_c = Wqkv[:, 0 * D + qcols].astype(bf)
        Wk_c = Wqkv[:, 1 * D + qcols].astype(bf)
        Wv_c = Wqkv[:, 2 * D + g * HCOLS : 2 * D + (g + 1) * HCOLS].astype(bf)
        Wp_c = Wproj[g * HCOLS : (g + 1) * HCOLS, :].astype(bf)
        in_maps.append(
            {"xT": xT[b], "Wq": Wq_c, "Wk": Wk_c, "Wv": Wv_c, "Wp": Wp_c}
        )

    results = _get_runner().run(in_maps)
    out = np.zeros((B, S, D), dtype=np.float32)
    for c in range(8):
        out[c // 4] += results[c]["out"]
    return out



# revision 2
# speedup vs baseline: 1.1902x; 1.1902x over previous
"""Trainium2 Bass kernel for nn_MultiHeadAttention (dense transformer block:
qkv proj + RoPE + causal SDPA + out proj), tensor-parallel over (batch, heads)
across 8 NeuronCores.

Sharding: 2 batches x 16 heads = 32 (b,h) pairs; core c handles batch c//4,
heads 4*(c%4)..4*(c%4)+3. Each core computes qkv for its 4 heads (from the
full x of its batch), RoPE, causal attention, and a PARTIAL output
projection (its heads' rows of Wproj); the host sums the 4 partials per
batch.

v2: all four big GEMMs (q/k/v projections and the output projection) run in
fp8e4m3 DoubleRow mode (2 contraction tiles per instruction, 0.5 cyc/row)
with a 3-term hi/lo error-compensated split:
    x @ W ~= (XA@WA + XA@WB + XC@WC) / 1024
where XA=fp8(x), XC=fp8(16*(x-XA)), WA=fp8(16*c*W), WC=fp8(c*W),
WB=fp8(16*(c*W - WC)), c=64. This keeps quantization error at bf16 level
(~0.1% per GEMM) while running the PE 1.33x faster than bf16 on those GEMMs.
Attention (scores / exp / PV) stays bf16.

Layout notes:
- x is passed pre-transposed per batch and pre-split into fp8 hi/lo (XA/XC
  [D, S]) so the contraction dim lands on SBUF partitions, streamed through
  SBUF in 512-token chunks.
- q/k head dims are permuted host-side into a 16-interleaved (even,odd)
  order so RoPE's pair swap is a quadrant-local DVE stream_shuffle.
  Attention scores are invariant to this (q and k permuted identically).
- RoPE tables carry sqrt(scale)/1024 (the fp8 descale); V stays at 1024x in
  bf16 and the 1/1024 is folded into the host-side Wproj scaling.
- Scores are computed transposed (S^T [kv, q]) so softmax's denominator
  comes from a ones-matmul (column sums) and P^T feeds the O = V^T @ P^T
  matmul directly. exp() runs without max-subtraction: |scores| < ~10 for
  this input distribution, safe in fp32.
- The attention output is renormalized into T1 = 32*y_head (ones matrix
  holds 32.0 so rinv = 1/(32*l)), then split into fp8 hi/lo (YA/YC) feeding
  the fp8 out-projection; final psum carries 32768*out, descaled by the
  ACT copy.
"""
import sys

sys.path.insert(0, "/opt/trn_rl_repo")

import numpy as np
import ml_dtypes

import concourse.bass as bass
import concourse.mybir as mybir
import concourse.tile as tile

P = 128
B, S, D = 2, 2048, 2048
NH, HD = 16, 128
NH_CORE = 4  # heads per core
HCOLS = NH_CORE * HD  # 512
KT = D // P  # 16 k-tiles
KP = KT // 2  # 8 k-tile pairs (DoubleRow)
TT = S // P  # 16 token tiles
QC = 512  # q-chunk width
NQC = S // QC  # 4
ROPE_THETA = 10000.0
SCALE = HD**-0.5
NEG = -30000.0

F32 = mybir.dt.float32
BF16 = mybir.dt.bfloat16
F8 = mybir.dt.float8e4
DR = mybir.MatmulPerfMode.DoubleRow

F8NP = ml_dtypes.float8_e4m3

# fp8 scale bookkeeping:
#   qkv:   XA(1) @ WA(1024) etc -> psum = 1024 * qkv
#   rope tables carry sqrt(SCALE)/1024  -> Qt/Kt = sqrt(SCALE) * q/k
#   Vt (bf16) = 1024 * v
#   ones matrix = 32.0 -> rinv = 1/(32*l) -> T1 = o_ps*rinv = 32*y_head
#   outproj: YA(32) @ WpA(1024/32*16... ) -> psum = 32768*out
OUT_DESCALE = 1.0 / 32768.0

_SWAP16 = [(i + 16) % 32 for i in range(32)]


# ---------------------------------------------------------------------------
# host-side constant tables
# ---------------------------------------------------------------------------
def _dim_perm():
    """Permutation p -> original head-dim index, 16-interleaved even/odd."""
    perm = np.zeros(HD, dtype=np.int64)
    for p in range(HD):
        qd, sl = p // 32, p % 32
        i = 16 * qd + (sl % 16)
        perm[p] = 2 * i if sl < 16 else 2 * i + 1
    return perm


def _rope_tables():
    """ctab[p,t], stab[p,t] (sign-baked) for the permuted head-dim layout."""
    perm = _dim_perm()
    inv_freq = 1.0 / (ROPE_THETA ** (np.arange(0, HD, 2, dtype=np.float64) / HD))
    t = np.arange(S, dtype=np.float64)
    ctab = np.zeros((HD, S), dtype=np.float64)
    stab = np.zeros((HD, S), dtype=np.float64)
    for p in range(HD):
        qd, sl = p // 32, p % 32
        i = 16 * qd + (sl % 16)
        ang = t * inv_freq[i]
        ctab[p] = np.cos(ang)
        stab[p] = -np.sin(ang) if sl < 16 else np.sin(ang)
    return ctab.astype(np.float32), stab.astype(np.float32)


def _tri_mask():
    """[P, P] f32: 0 where kv(row) <= q(col) else NEG."""
    b = np.arange(P)[:, None]
    a = np.arange(P)[None, :]
    return np.where(b <= a, 0.0, NEG).astype(np.float32)


def _split3_w(W, c):
    """3-term fp8 split of weights: WA=fp8(16c*W), WB=fp8(16*(c*W-fp8(c*W))),
    WC=fp8(c*W). All numpy fp8e4m3."""
    Ws = (c * W).astype(np.float32)
    WC = Ws.astype(F8NP)
    WB = (16.0 * (Ws - WC.astype(np.float32))).astype(F8NP)
    WA = (16.0 * Ws).astype(F8NP)
    return WA, WB, WC


def _split2_x(x):
    """XA=fp8(x), XC=fp8(16*(x-XA))."""
    XA = x.astype(F8NP)
    XC = (16.0 * (x - XA.astype(np.float32))).astype(F8NP)
    return XA, XC


# ---------------------------------------------------------------------------
# device kernel
# ---------------------------------------------------------------------------
def _build_nc():
    nc = bass.Bass()

    XA = nc.declare_dram_parameter("XA", [D, S], F8, isOutput=False)
    XC = nc.declare_dram_parameter("XC", [D, S], F8, isOutput=False)
    wq = [nc.declare_dram_parameter(f"Wq{t}", [D, HCOLS], F8, isOutput=False)
          for t in "ABC"]
    wk = [nc.declare_dram_parameter(f"Wk{t}", [D, HCOLS], F8, isOutput=False)
          for t in "ABC"]
    wv = [nc.declare_dram_parameter(f"Wv{t}", [D, HCOLS], F8, isOutput=False)
          for t in "ABC"]
    wp = [nc.declare_dram_parameter(f"Wp{t}", [HCOLS, D], F8, isOutput=False)
          for t in "ABC"]
    out = nc.declare_dram_parameter("out", [S, D], F32, isOutput=True)

    # sqrt(SCALE)/1024 on both q and k tables => scores scaled by SCALE
    ctab_np, stab_np = _rope_tables()
    rt = np.float32(np.sqrt(SCALE) / 1024.0)
    cq_d = nc.inline_tensor((ctab_np * rt).astype(ml_dtypes.bfloat16), "cq")
    sq_d = nc.inline_tensor((stab_np * rt).astype(ml_dtypes.bfloat16), "sq")
    mask_d = nc.inline_tensor(_tri_mask(), "trimask")

    XA_t = XA[:].rearrange("(ko p) t -> p ko t", p=P)
    XC_t = XC[:].rearrange("(ko p) t -> p ko t", p=P)
    wq_t = [w[:].rearrange("(ko p) m -> p ko m", p=P) for w in wq]
    wk_t = [w[:].rearrange("(ko p) m -> p ko m", p=P) for w in wk]
    wv_t = [w[:].rearrange("(ko p) m -> p ko m", p=P) for w in wv]
    wp_t = [w[:].rearrange("(ho p) n -> p ho n", p=P) for w in wp]
    out_t = out[:].rearrange("(to p) n -> p to n", p=P)

    with tile.TileContext(nc) as tc:
        with (
            tc.tile_pool(name="persist", bufs=1) as pp,
            tc.tile_pool(name="work", bufs=2) as wkp,
            tc.tile_pool(name="xwin", bufs=3) as xw,
        ):
            # persistent tiles
            cq = pp.tile([P, S], BF16)
            sq = pp.tile([P, S], BF16)
            nc.sync.dma_start(cq, cq_d[:])
            nc.sync.dma_start(sq, sq_d[:])
            trimask = pp.tile([P, P], F32)
            nc.sync.dma_start(trimask, mask_d[:])
            ones_sb = pp.tile([P, P], BF16)
            nc.vector.memset(ones_sb, 32.0)

            Qt = pp.tile([P, NH_CORE, S], BF16)
            Kt = pp.tile([P, NH_CORE, S], BF16)
            Vt = pp.tile([P, TT, HCOLS], BF16)
            YA = pp.tile([P, NH_CORE, S], F8)
            YC = pp.tile([P, NH_CORE, S], F8)

            # ---------------- phase 1: qkv projection + RoPE ---------------
            with (
                tc.tile_pool(name="wsb", bufs=1) as wsb,
                tc.tile_pool(name="ps_mm1", bufs=8, space="PSUM") as psA,
            ):
                wq_sb = [wsb.tile([P, KT, HCOLS], F8, name=f"wq{t}")
                         for t in "ABC"]
                wk_sb = [wsb.tile([P, KT, HCOLS], F8, name=f"wk{t}")
                         for t in "ABC"]
                wv_sb = [wsb.tile([P, KT, HCOLS], F8, name=f"wv{t}")
                         for t in "ABC"]
                for i in range(3):
                    nc.gpsimd.dma_start(wk_sb[i], wk_t[i])
                    nc.gpsimd.dma_start(wv_sb[i], wv_t[i])
                    nc.gpsimd.dma_start(wq_sb[i], wq_t[i])

                def load_xchunk(tcx):
                    xa = xw.tile([P, KT, QC], F8, tag="xa")
                    xc = xw.tile([P, KT, QC], F8, tag="xc")
                    nc.sync.dma_start(xa, XA_t[:, :, tcx * QC:(tcx + 1) * QC])
                    nc.sync.dma_start(xc, XC_t[:, :, tcx * QC:(tcx + 1) * QC])
                    return xa, xc

                def mm3(ps, w3, x2, lhs_w, hsl, xsl):
                    """24 DoubleRow matmuls accumulating the 3-term product.
                    lhs_w: True if weights are the stationary operand."""
                    terms = ((w3[0], x2[0]), (w3[1], x2[0]), (w3[2], x2[1]))
                    n = len(terms) * KP
                    i = 0
                    for wt, xt in terms:
                        for kp in range(KP):
                            ks = slice(2 * kp, 2 * kp + 2)
                            if lhs_w:
                                lhsT = wt[:, ks, hsl]
                                rhs = xt[:, ks, xsl]
                            else:
                                lhsT = xt[:, ks, xsl]
                                rhs = wt[:, ks, hsl]
                            nc.tensor.matmul(
                                ps, lhsT, rhs,
                                start=(i == 0), stop=(i == n - 1),
                                perf_mode=DR,
                            )
                            i += 1

                xa_c, xc_c = load_xchunk(0)
                for tcx in range(NQC):
                    if tcx + 1 < NQC:
                        xa_n, xc_n = load_xchunk(tcx + 1)
                    csl = slice(tcx * QC, (tcx + 1) * QC)
                    # q and k projections with fused RoPE
                    for w3, O_t in ((wk_sb, Kt), (wq_sb, Qt)):
                        for h in range(NH_CORE):
                            ps = psA.tile([P, QC], F32, tag="ps")
                            mm3(ps, w3, (xa_c, xc_c), True,
                                slice(h * HD, (h + 1) * HD), slice(None))
                            pc = wkp.tile([P, QC], BF16, tag="pc")
                            nc.scalar.activation(
                                pc, ps, mybir.ActivationFunctionType.Copy
                            )
                            xsw = wkp.tile([P, QC], BF16, tag="xsw")
                            nc.vector.stream_shuffle(xsw, pc, _SWAP16)
                            m1 = wkp.tile([P, QC], BF16, tag="m1")
                            nc.vector.tensor_mul(m1, pc, cq[:, csl])
                            m2 = wkp.tile([P, QC], BF16, tag="m2")
                            nc.vector.tensor_mul(m2, xsw, sq[:, csl])
                            nc.vector.tensor_add(O_t[:, h, csl], m1, m2)
                    # v projection (token-major); Vt stays at 1024x scale
                    for sub in range(4):
                        tt = 4 * tcx + sub
                        ps = psA.tile([P, HCOLS], F32, tag="ps")
                        mm3(ps, wv_sb, (xa_c, xc_c), False,
                            slice(None), slice(sub * P, (sub + 1) * P))
                        nc.scalar.activation(
                            Vt[:, tt], ps, mybir.ActivationFunctionType.Copy
                        )
                    if tcx + 1 < NQC:
                        xa_c, xc_c = xa_n, xc_n

            # ---------------- phase 2: attention ---------------------------
            with (
                tc.tile_pool(name="wp", bufs=1) as wpp,
            ):
                wp_sb = [wpp.tile([P, NH_CORE, D], F8, name=f"wp{t}")
                         for t in "ABC"]
                for i in range(3):
                    nc.sync.dma_start(wp_sb[i], wp_t[i])

                with (
                    tc.tile_pool(name="pt", bufs=3) as ptp,
                    tc.tile_pool(name="t1", bufs=2) as t1p,
                    tc.tile_pool(name="ps_o", bufs=2, space="PSUM") as psO,
                    tc.tile_pool(name="ps_s", bufs=5, space="PSUM") as psS,
                    tc.tile_pool(name="ps_l", bufs=1, space="PSUM") as psL,
                ):
                  for h in range(NH_CORE):
                    for qc in range(NQC):
                        o_ps = psO.tile([P, QC], F32, tag="ops")
                        l_ps = psL.tile([P, QC], F32, tag="lps")
                        njb = 4 * qc + 4
                        for jb in range(njb):
                            d = jb - 4 * qc  # diag offset if >= 0
                            off = 128 * d if d > 0 else 0
                            s_ps = psS.tile([P, QC], F32, tag="sps")
                            nc.tensor.matmul(
                                s_ps[:, off:],
                                Kt[:, h, jb * P:(jb + 1) * P],
                                Qt[:, h, qc * QC + off:(qc + 1) * QC],
                                start=True,
                                stop=True,
                            )
                            if d >= 0:
                                nc.vector.tensor_add(
                                    s_ps[:, off:off + P],
                                    s_ps[:, off:off + P],
                                    trimask,
                                )
                            pt = ptp.tile([P, QC], BF16, tag="pt")
                            nc.scalar.activation(
                                pt[:, off:],
                                s_ps[:, off:],
                                mybir.ActivationFunctionType.Exp,
                            )
                            nc.tensor.matmul(
                                o_ps[:, off:],
                                Vt[:, jb, h * HD:(h + 1) * HD],
                                pt[:, off:],
                                start=(jb == 0),
                                stop=(jb == njb - 1),
                            )
                            nc.tensor.matmul(
                                l_ps[:, off:],
                                ones_sb,
                                pt[:, off:],
                                start=(jb == 0),
                                stop=(jb == njb - 1),
                            )
                        qsl = slice(qc * QC, (qc + 1) * QC)
                        rinv = wkp.tile([P, QC], F32, tag="rinv")
                        nc.vector.reciprocal(rinv, l_ps)
                        # T1 = 32 * y_head (f32), then fp8 hi/lo split
                        t1 = t1p.tile([P, QC], F32, tag="t1")
                        nc.vector.tensor_mul(t1, o_ps, rinv)
                        nc.gpsimd.tensor_copy(YA[:, h, qsl], t1)
                        res = t1p.tile([P, QC], F32, tag="res")
                        nc.vector.tensor_sub(res, t1, YA[:, h, qsl])
                        nc.gpsimd.tensor_scalar_mul(YC[:, h, qsl], res, 16.0)

                # ------------- phase 3: fp8 out projection ----------------
                with (
                    tc.tile_pool(name="outp", bufs=3) as outp,
                    tc.tile_pool(name="ps_p", bufs=6, space="PSUM") as psP,
                ):
                    for tt in range(TT):
                        ob = outp.tile([P, D], F32, tag="ob")
                        tsl = slice(tt * P, (tt + 1) * P)
                        for ncx in range(D // QC):
                            nsl = slice(ncx * QC, (ncx + 1) * QC)
                            ps = psP.tile([P, QC], F32, tag="psp")
                            terms = ((YA, wp_sb[0]), (YA, wp_sb[1]),
                                     (YC, wp_sb[2]))
                            i = 0
                            for yt, wt in terms:
                                for hp in range(NH_CORE // 2):
                                    hs = slice(2 * hp, 2 * hp + 2)
                                    nc.tensor.matmul(
                                        ps,
                                        yt[:, hs, tsl],
                                        wt[:, hs, nsl],
                                        start=(i == 0),
                                        stop=(i == 5),
                                        perf_mode=DR,
                                    )
                                    i += 1
                            nc.scalar.activation(
                                ob[:, nsl], ps,
                                mybir.ActivationFunctionType.Copy,
                                scale=float(OUT_DESCALE),
                            )
                        eng = nc.sync if tt % 2 == 0 else nc.gpsimd
                        eng.dma_start(out_t[:, tt], ob)
    return nc


# ---------------------------------------------------------------------------
# legalization: this walrus build supports only ONE sync wait per instruction
# ---------------------------------------------------------------------------
_ENGINE_SEM_PREFIX = {
    "PE": "PE_",
    "DVE": "DVE_",
    "ACT": "ACT_",
    "Pool": "POOL_",
    "SP": "SP_",
}
_wf_counter = [0]


def _legalize(nc, max_waits=1):
    for f in nc.m.functions:
        for bb in f.blocks:
            new_insts = []
            for inst in bb.instructions:
                si = getattr(inst, "sync_info", None)
                eng = getattr(inst, "engine", None)
                if si is None or not si.on_wait or eng is None:
                    new_insts.append(inst)
                    continue
                waits = list(si.on_wait)
                pref = _ENGINE_SEM_PREFIX.get(eng.name)
                if pref is not None:
                    waits = [
                        w
                        for w in waits
                        if not (
                            w.sync_type == "semaphore"
                            and w.ant_name.startswith(pref)
                        )
                    ]
                if len(waits) > max_waits:
                    for w in waits[:-max_waits]:
                        _wf_counter[0] += 1
                        nop = mybir.InstNoOp(
                            name=f"I-waitfix-{_wf_counter[0]}", ins=[], outs=[]
                        )
                        nop.engine = eng
                        nop.sync_info = mybir.SyncInfo(on_wait=[w], on_update=[])
                        new_insts.append(nop)
                    waits = waits[-max_waits:]
                if len(waits) != len(si.on_wait):
                    inst.sync_info = mybir.SyncInfo(
                        on_wait=waits, on_update=list(si.on_update)
                    )
                new_insts.append(inst)
            bb.instructions[:] = new_insts


# ---------------------------------------------------------------------------
# SPMD runner (mirrors concourse.bass2jax.run_bass_via_pjrt, kept resident)
# ---------------------------------------------------------------------------
class _Runner:
    def __init__(self, nc, n_cores=8):
        import jax
        from jax.sharding import Mesh, PartitionSpec
        from jax.experimental.shard_map import shard_map
        from concourse import bass2jax
        from concourse.bass2jax import _bass_exec_p, install_neuronx_cc_hook

        install_neuronx_cc_hook()
        self.jax = jax
        self.nc = nc
        self.n_cores = n_cores
        partition_name = (
            nc.partition_id_tensor.name if nc.partition_id_tensor else None
        )
        in_names, out_names, out_avals, zero_outs = [], [], [], []
        for alloc in nc.m.functions[0].allocations:
            if not isinstance(alloc, mybir.MemoryLocationSet):
                continue
            name = alloc.memorylocations[0].name
            if alloc.kind == "ExternalInput":
                if name != partition_name:
                    in_names.append(name)
            elif alloc.kind == "ExternalOutput":
                shape = tuple(alloc.tensor_shape)
                dtype = mybir.dt.np(alloc.dtype)
                out_names.append(name)
                out_avals.append(jax.core.ShapedArray(shape, dtype))
                zero_outs.append(np.zeros(shape, dtype))
        self.in_names, self.out_names = in_names, out_names
        self.out_avals, self.zero_outs = out_avals, zero_outs
        n_params, n_outs = len(in_names), len(out_names)
        all_in_names = in_names + out_names
        if partition_name is not None:
            all_in_names.append(partition_name)
        donate = tuple(range(n_params, n_params + n_outs))

        def _body(*args):
            operands = list(args)
            if partition_name is not None:
                operands.append(bass2jax.partition_id_tensor())
            return tuple(
                _bass_exec_p.bind(
                    *operands,
                    out_avals=tuple(out_avals),
                    in_names=tuple(all_in_names),
                    out_names=tuple(out_names),
                    lowering_input_output_aliases=(),
                    sim_require_finite=True,
                    sim_require_nnan=True,
                    nc=nc,
                )
            )

        devices = jax.devices()[:n_cores]
        mesh = Mesh(np.asarray(devices), ("core",))
        in_specs = (PartitionSpec("core"),) * (n_params + n_outs)
        out_specs = (PartitionSpec("core"),) * n_outs
        self.fn = jax.jit(
            shard_map(
                _body,
                mesh=mesh,
                in_specs=in_specs,
                out_specs=out_specs,
                check_rep=False,
            ),
            donate_argnums=donate,
            keep_unused=True,
        )

    def run(self, in_maps):
        n = self.n_cores
        concat_in = [
            np.concatenate(
                [np.asarray(in_maps[c][name]) for c in range(n)], axis=0
            )
            for name in self.in_names
        ]
        zeros = [
            np.zeros((n * z.shape[0], *z.shape[1:]), z.dtype)
            for z in self.zero_outs
        ]
        out_arrs = self.fn(*concat_in, *zeros)
        return [
            {
                name: np.asarray(out_arrs[i]).reshape(
                    n, *self.out_avals[i].shape
                )[c]
                for i, name in enumerate(self.out_names)
            }
            for c in range(n)
        ]


_RUNNER = None


def _get_runner():
    global _RUNNER
    if _RUNNER is None:
        nc = _build_nc()
        _legalize(nc)
        _RUNNER = _Runner(nc, 8)
    return _RUNNER


# ---------------------------------------------------------------------------
# public entry point
# ---------------------------------------------------------------------------
def kernel(x, Wqkv, Wproj):
    x = np.asarray(x, dtype=np.float32)
    Wqkv = np.asarray(Wqkv, dtype=np.float32)
    Wproj = np.asarray(Wproj, dtype=np.float32)
    perm = _dim_perm()

    xsplit = [_split2_x(np.ascontiguousarray(x[b].T)) for b in range(B)]
    in_maps = []
    for c in range(8):
        b, g = c // 4, c % 4
        heads = range(NH_CORE * g, NH_CORE * (g + 1))
        qcols = np.concatenate([h * HD + perm for h in heads])
        WqA, WqB, WqC = _split3_w(Wqkv[:, 0 * D + qcols], 64.0)
        WkA, WkB, WkC = _split3_w(Wqkv[:, 1 * D + qcols], 64.0)
        WvA, WvB, WvC = _split3_w(
            Wqkv[:, 2 * D + g * HCOLS: 2 * D + (g + 1) * HCOLS], 64.0
        )
        # Wp: T1 = 32*y_head; fold 1/32 here. c=2048 keeps fp8 in normal range
        WpA, WpB, WpC = _split3_w(
            Wproj[g * HCOLS:(g + 1) * HCOLS, :] / 32.0, 2048.0
        )
        xa, xc = xsplit[b]
        in_maps.append({
            "XA": xa, "XC": xc,
            "WqA": WqA, "WqB": WqB, "WqC": WqC,
            "WkA": WkA, "WkB": WkB, "WkC": WkC,
            "WvA": WvA, "WvB": WvB, "WvC": WvC,
            "WpA": WpA, "WpB": WpB, "WpC": WpC,
        })

    results = _get_runner().run(in_maps)
    out = np.zeros((B, S, D), dtype=np.float32)
    for c in range(8):
        out[c // 4] += results[c]["out"]
    return out
